# revision 67
# baseline (speedup 1.0000x reference)
"""CTRNN policy kernel for Trainium2 (8 NeuronCores, batch-parallel).

Reference computation (per batch element b, B=64, N=1024, OBS=64, A=16):
    I = E[b] @ obs[b]
    repeat int(1.0//0.1)=9 times:
        y = tanh(gain*(v+bias))*mask
        v = (v + DT/tau * (-v + W[b]@y + I)) * mask
    action[b] = D[b] @ v

Sharding: batch 64 -> 8 cores x 8 individuals, fully data parallel.

Per-core algebra (host-folded, mask/tau folded into the coefficients):
    am = DT/tau*mask, cm = (1-DT/tau)*mask
    Wf = diag(am) @ W @ diag(mask);  Ef = diag(am) @ E;  bc = bias*(1-cm)
    state vs = v + bias:
        y   = tanh(g * vs)
        vs' = cm*vs + Wf@y + (Ef@obs + bc)
    action = D @ (vs - bias)

Device mapping per individual (N=1024 as n = p*8 + c for the matmul
contraction; W^T slabs [128, 8192] bf16 all resident in SBUF):

  - matvec on TensorE with 4-way column-group tiling: stationary = y column
    chunk [128,1] bf16 at array column 32j, moving = W^T n-slab [128,256].
    The 4 groups stream concurrently (separate XBUSes) and land in ONE
    shared PSUM bank at partitions {0,32,64,96} (disjoint per-partition
    accumulators), so a matvec costs ~1.9us of PE instead of ~3.5us.
  - the leak/gate update runs in "row space" [128,256] right out of PSUM
    (rows 32j hold dv n-slab j; other lanes carry zeros): tensor_tensor ops
    are lane-parallel so the garbage lanes are free. The only partition
    redistribution is the y scatter [4x256 rows] -> [128,8] bf16 column
    layout, issued at the END of the chain on the ACT HWDGE ring: its ~1.5us
    DMA completion latency is absorbed by the 6-wide round-robin before the
    same individual's next matvec needs y -- no engine FIFO ever waits on a
    DMA completion (that coupling capped earlier versions at ~2.9us/matvec).
"""

import os
import sys
from contextlib import ExitStack

import numpy as np

for _p in ("/opt/trn_rl_repo", "/root/.axon_site/_ro/trn_rl_repo"):
    if os.path.isdir(_p) and _p not in sys.path:
        sys.path.append(_p)

import ml_dtypes  # noqa: E402

import concourse.bass as bass  # noqa: E402
import concourse.tile as tile  # noqa: E402
from concourse import bacc, mybir  # noqa: E402
from concourse.bass_utils import run_bass_kernel_spmd  # noqa: E402

DT = 0.1
ITERS = int(1.0 // DT)  # == 9: 1.0//0.1 == 9.0 in fp
B_FULL, N, OBS, ADIM = 64, 1024, 64, 16
NCORES = 8
BPC = B_FULL // NCORES
P, CN = 128, 8          # n = p*8 + c
F32 = mybir.dt.float32
BF16 = mybir.dt.bfloat16
BF16_NP = ml_dtypes.bfloat16

CG = 4                  # column groups
NSLAB = N // CG         # 256
GSLABS = [(32 * j, 256 * j) for j in range(CG)]  # (tile_col, w_off)
PSW = 512               # one PSUM bank per matvec
PS_BUFS = 8

# contraction chunk map for the DVE-block-transpose y distribution:
# chunk k's stationary column is yT[:, 32k] where yT = 32x32-block-transpose
# of row-space y4 (rows 32j hold n-slab j). That column holds
# y[m_k(p)] with m_k(p) = 256*(p//32) + 32*k + (p%32); the host permutes the
# W^T slabs to match, so no DMA scatter is needed anywhere in the loop.
_pidx = np.arange(P)
M_INDEX = (256 * (_pidx[:, None] // 32) + 32 * np.arange(CN)[None, :]
           + (_pidx[:, None] % 32))  # [128, 8]


def wave_schedule(iters=ITERS, mv_us=2.0):
    """Greedy longest-queue-first rounds honoring estimated W arrival.

    Returns rounds (lists of (b, t) work items). Width grows as W tiles land
    (~4.8us each after W0+bulk), then stays ~6-wide so the per-individual
    y chain (incl. scatter DMA latency) hides behind other individuals.
    """
    w_avail = [8.0, 26.0, 32.0, 40.0, 48.0, 56.0, 64.0, 72.0]
    t = 9.0
    remaining = [iters] * BPC
    rounds = []
    while any(remaining):
        active = [b for b in range(BPC) if remaining[b] and w_avail[b] <= t]
        if not active:
            t = min(w_avail[b] for b in range(BPC) if remaining[b])
            continue
        active.sort(key=lambda b: (-remaining[b], b))
        active = active[:6]
        rounds.append([(b, iters - remaining[b]) for b in active])
        for b in active:
            remaining[b] -= 1
        t += max(mv_us * len(active), 8.0 if len(active) < 4 else 0.0)
    return rounds


def make_pools(ctx, tc):
    return dict(
        const=ctx.enter_context(tc.tile_pool(name="const", bufs=1)),
        wpool=ctx.enter_context(tc.tile_pool(name="w", bufs=1)),
        et=ctx.enter_context(tc.tile_pool(name="et", bufs=2)),
        row=ctx.enter_context(tc.tile_pool(name="row", bufs=2)),
        tmp=ctx.enter_context(tc.tile_pool(name="tmp", bufs=4)),
        t1p=ctx.enter_context(tc.tile_pool(name="t1p", bufs=8)),
        ps=ctx.enter_context(tc.tile_pool(name="ps", bufs=PS_BUFS, space="PSUM")),
    )


def kernel_body(ctx, tc, ins, out_ap, iters=ITERS, pools=None, cm_const=None):
    nc = tc.nc
    Tanh = mybir.ActivationFunctionType.Tanh
    add = mybir.AluOpType.add
    mult = mybir.AluOpType.mult
    sub = mybir.AluOpType.subtract

    p = pools if pools is not None else make_pools(ctx, tc)
    const, wpool, etp, row, tmp, ps = (
        p["const"], p["wpool"], p["et"], p["row"], p["tmp"], p["ps"])
    t1p = p["t1p"]

    # ---- bulk loads on the SP (sync) HWDGE ring: W first ----
    w_sb = [wpool.tile([P, CN * N], BF16, tag=f"w{b}", name=f"w{b}")
            for b in range(BPC)]
    obs_sb = const.tile([OBS, BPC], BF16, tag="obs", name="obs")
    nc.sync.dma_start(obs_sb[:], ins["obsT"][:])
    bias_c = const.tile([P, BPC * 2 * CN], F32, tag="biasc", name="biasc")
    nc.sync.dma_start(bias_c[:], ins["biascol"][:])
    gbrow = const.tile([P, BPC * 2 * NSLAB], F32, tag="gbrow", name="gbrow")
    nc.sync.dma_start(gbrow[:, :3 * 2 * NSLAB], ins["gbrow"][:, :3 * 2 * NSLAB])
    vs4 = [const.tile([P, NSLAB], BF16, tag=f"vs{b}", name=f"vs{b}")
           for b in range(BPC)]
    for b in range(3):
        nc.sync.dma_start(vs4[b][:], ins["vs0row"][:, b * NSLAB:(b + 1) * NSLAB])
    nc.sync.dma_start(w_sb[0][:], ins["WT"][0])
    nc.sync.dma_start(w_sb[1][:], ins["WT"][1])
    nc.sync.dma_start(gbrow[:, 3 * 2 * NSLAB:], ins["gbrow"][:, 3 * 2 * NSLAB:])
    for b in range(3, BPC):
        nc.sync.dma_start(vs4[b][:], ins["vs0row"][:, b * NSLAB:(b + 1) * NSLAB])
    nc.sync.dma_start(w_sb[2][:], ins["WT"][2])
    cmrow = None
    if cm_const is None:
        cmrow = const.tile([P, BPC * NSLAB], F32, tag="cmrow", name="cmrow")
        nc.sync.dma_start(cmrow[:], ins["cmrow"][:])
    dt_sb = const.tile([P, BPC * CN * ADIM], F32, tag="dt", name="dt")
    nc.sync.dma_start(dt_sb[:], ins["DTall"][:])
    for b in range(3, BPC):
        nc.sync.dma_start(w_sb[b][:], ins["WT"][b])

    i4_sb, y_sb = [], []
    for b in range(BPC):
        i4_sb.append(const.tile([P, NSLAB], F32, tag=f"i4{b}", name=f"i4{b}"))
        y_sb.append(const.tile([P, NSLAB], BF16, tag=f"y{b}", name=f"y{b}"))
    act_sb = const.tile([1, BPC * ADIM], F32, tag="act", name="act")

    def cm4_ap(b):
        return cmrow[:, b * NSLAB:(b + 1) * NSLAB]

    def g4_ap(b):
        return gbrow[:, (b * 2 + 0) * NSLAB:(b * 2 + 1) * NSLAB]

    def bc4_ap(b):
        return gbrow[:, (b * 2 + 1) * NSLAB:(b * 2 + 2) * NSLAB]

    # gate + y distribution: s4(bf16) -> DVE 32x32 block-transpose -> tanh.
    # tanh(transpose(x)) == transpose(tanh(x)), and transposing first keeps
    # the vector->vector hop adjacent with ScalarE writing y_sb[b] directly;
    # yT[:, 32k] is chunk k's stationary column. No DMA anywhere.
    def emit_y(b, s4):
        sT = tmp.tile([P, NSLAB], BF16, tag="y4", name="y4")
        nc.vector.transpose(sT[:], s4[:])
        nc.scalar.activation(y_sb[b][:], sT[:], Tanh)

    # ---- setup: I row = Ef@obs, scatter to row space, add bc; y0 ----
    for b in range(BPC):
        ir = row.tile([1, N], F32, tag="irow", name=f"ir{b}")
        et = etp.tile([OBS, N], BF16, tag="et", name="et")
        # ACT-ring DMA: must not queue behind the W stream on the SP ring
        nc.scalar.dma_start(et[:], ins["ETall"][:, b * N:(b + 1) * N])
        for h in range(2):
            ip = ps.tile([P, PSW], F32, tag="ps", name=f"ip{b}_{h}")
            nc.tensor.matmul(
                ip[0:1, 0:512],
                obs_sb[:, b:b + 1],
                et[:, h * 512:(h + 1) * 512],
                start=True, stop=True,
            )
            nc.scalar.copy(ir[0:1, h * 512:(h + 1) * 512], ip[0:1, 0:512])
        nc.scalar.dma_start(i4_sb[b][0:P:32, :], ir[:])  # [1,1024]->[4,256] rows
        nc.vector.tensor_tensor(i4_sb[b][:], i4_sb[b][:], bc4_ap(b), op=add)
        emit_y(b, vs4[b])

    # ---- recurrent loop ----
    t1_sb, wy_sb = {}, {}

    def matvec_mm(b, t):
        # t1 = cm*vs + I only needs last iteration's vs -- runs during the
        # matmuls, off the post-matmul chain. With constant tau/mask the
        # leak multiply is a ScalarE const-mul, keeping VectorE under PE.
        tg = tmp.tile([P, NSLAB], F32, tag="tg", name="tg")
        if cm_const is not None:
            nc.scalar.mul(tg[:], vs4[b][:], cm_const)
        else:
            nc.vector.tensor_tensor(tg[:], cm4_ap(b), vs4[b][:], op=mult)
        t1 = t1p.tile([P, NSLAB], F32, tag="t1", name="t1")
        nc.vector.tensor_tensor(t1[:], tg[:], i4_sb[b][:], op=add)
        t1_sb[b] = t1
        wy = ps.tile([P, PSW], F32, tag="ps", name="wy")
        wy_sb[b] = wy
        for c in range(CN):
            yc = y_sb[b][:, 32 * c:32 * c + 1]
            for (tcol, woff) in GSLABS:
                nc.tensor.matmul(
                    wy[tcol:tcol + 1, 0:NSLAB],
                    yc,
                    w_sb[b][:, c * N + woff: c * N + woff + NSLAB],
                    start=(c == 0), stop=(c == CN - 1),
                    tile_position=(0, tcol),
                )

    def matvec_upd(b, t):
        # row-space leak/gate update straight out of PSUM
        nc.vector.tensor_tensor(vs4[b][:], wy_sb[b][:, 0:NSLAB], t1_sb[b][:],
                                op=add)
        if t < iters - 1:
            emit_y(b, vs4[b])

    # ---- decode: action = D @ (vs - bias) ----
    vcol_sb = {}

    def decode_pre(b):
        # issued right after b's last update; completes long before the
        # end-of-kernel decode matmuls so they never stall PE
        vcol = const.tile([P, CN], BF16, tag=f"vc{b}", name=f"vc{b}")
        nc.scalar.dma_start(vcol[:], vs4[b][0:P:32, :])
        vcol_sb[b] = vcol

    def decode(b):
        vg = tmp.tile([P, CN], F32, tag="vg", name="vg")
        nc.vector.tensor_tensor(
            vg[:], vcol_sb[b][:], bias_c[:, 2 * b * CN:(2 * b + 1) * CN],
            op=mult)
        vf = tmp.tile([P, CN], F32, tag="vf", name="vf")
        nc.vector.tensor_tensor(
            vf[:], vg[:], bias_c[:, (2 * b + 1) * CN:(2 * b + 2) * CN], op=sub)
        ap = ps.tile([P, PSW], F32, tag="ps", name="dec")
        for c in range(CN):
            nc.tensor.matmul(
                ap[0:1, 0:ADIM],
                vf[:, c:c + 1],
                dt_sb[:, b * CN * ADIM + c * ADIM: b * CN * ADIM + (c + 1) * ADIM],
                start=(c == 0), stop=(c == CN - 1),
            )
        nc.vector.tensor_copy(act_sb[0:1, b * ADIM:(b + 1) * ADIM], ap[0:1, 0:ADIM])

    for round_items in wave_schedule(iters):
        for b, t in round_items:
            matvec_mm(b, t)
        for b, t in round_items:
            matvec_upd(b, t)
            if t == iters - 1:
                decode_pre(b)
    for b in range(BPC):
        decode(b)
    nc.sync.dma_start(out_ap[:], act_sb[0:1, :])


def build_nc(iters=ITERS, cm_const=None):
    nc = bacc.Bacc(
        "TRN2", target_bir_lowering=False, debug=False, enable_asserts=False,
    )
    ins = {}
    ins["WT"] = nc.dram_tensor("WT", [BPC, P, CN * N], BF16, kind="ExternalInput").ap()
    ins["ETall"] = nc.dram_tensor("ETall", [OBS, BPC * N], BF16, kind="ExternalInput").ap()
    ins["obsT"] = nc.dram_tensor("obsT", [OBS, BPC], BF16, kind="ExternalInput").ap()
    ins["gbrow"] = nc.dram_tensor(
        "gbrow", [P, BPC * 2 * NSLAB], F32, kind="ExternalInput").ap()
    ins["cmrow"] = nc.dram_tensor(
        "cmrow", [P, BPC * NSLAB], F32, kind="ExternalInput").ap()
    ins["vs0row"] = nc.dram_tensor(
        "vs0row", [P, BPC * NSLAB], BF16, kind="ExternalInput").ap()
    ins["biascol"] = nc.dram_tensor(
        "biascol", [P, BPC * 2 * CN], F32, kind="ExternalInput").ap()
    ins["DTall"] = nc.dram_tensor(
        "DTall", [P, BPC * CN * ADIM], F32, kind="ExternalInput").ap()
    out_ap = nc.dram_tensor("act", [BPC, ADIM], F32, kind="ExternalOutput").ap()

    with tile.TileContext(nc) as tc:
        with ExitStack() as ctx:
            pools = make_pools(ctx, tc)
            kernel_body(ctx, tc, ins, out_ap, iters, pools, cm_const)
    nc.compile()
    return nc


def _to_rowspace(arr):
    """[B, N] -> [B, 128, NSLAB] row-space: row 32j holds n-slab j, rest 0."""
    B = arr.shape[0]
    out = np.zeros((B, P, NSLAB), np.float32)
    for j in range(CG):
        out[:, 32 * j, :] = arr[:, NSLAB * j:NSLAB * (j + 1)]
    return out


def prep_in_maps(obs, v0, tau, gain, bias, W, mask, E, D):
    f = np.float32
    obs, v0, tau, gain, bias, W, mask, E, D = [
        np.asarray(x, dtype=f) for x in (obs, v0, tau, gain, bias, W, mask, E, D)
    ]
    am = (DT / tau) * mask                    # [64, N]
    cm = (1.0 - DT / tau) * mask
    Wf = W * (am * gain)[:, :, None] * mask[:, None, :]
    # permute the contraction dim to the block-transpose chunk map:
    # WT[b, p, k*N + n] = Wf[b, n, M_INDEX[p, k]]   (done per-core for memory)
    WT = np.empty((B_FULL, P, CN * N), BF16_NP)
    for b in range(B_FULL):
        WT[b] = Wf[b][:, M_INDEX].transpose(1, 2, 0).reshape(
            P, CN * N).astype(BF16_NP)
    ETp = np.ascontiguousarray(
        (E * (am * gain)[:, :, None]).transpose(0, 2, 1)).astype(BF16_NP)
    DTp = np.ascontiguousarray(D.transpose(0, 2, 1)).reshape(B_FULL, P, CN * ADIM)
    obsT = np.ascontiguousarray(obs.T).astype(BF16_NP)  # [OBS, 64]
    cm4 = _to_rowspace(cm)
    g4 = _to_rowspace(gain)
    bc4 = _to_rowspace(gain * bias * (1.0 - cm))
    vs04 = _to_rowspace(gain * (v0 + bias))
    invg = np.where(gain != 0.0, 1.0 / np.maximum(gain, 1e-30), 0.0)
    biascol = np.concatenate(
        [invg.reshape(B_FULL, P, 1, CN), bias.reshape(B_FULL, P, 1, CN)],
        axis=2).reshape(B_FULL, P, 2 * CN)

    in_maps = []
    for core in range(NCORES):
        s = slice(core * BPC, (core + 1) * BPC)
        gbrow = np.empty((P, BPC * 2 * NSLAB), f)
        cmrow = np.empty((P, BPC * NSLAB), f)
        for i, b in enumerate(range(core * BPC, (core + 1) * BPC)):
            for k, arr in enumerate((g4, bc4)):
                gbrow[:, (i * 2 + k) * NSLAB:(i * 2 + k + 1) * NSLAB] = arr[b]
            cmrow[:, i * NSLAB:(i + 1) * NSLAB] = cm4[b]
        vs0row = np.ascontiguousarray(
            vs04[s].transpose(1, 0, 2).reshape(P, BPC * NSLAB)).astype(BF16_NP)
        bcol = np.ascontiguousarray(
            biascol[s].transpose(1, 0, 2).reshape(P, BPC * 2 * CN))
        et = np.ascontiguousarray(
            ETp[s].transpose(1, 0, 2).reshape(OBS, BPC * N))
        dtall = np.ascontiguousarray(
            DTp[s].transpose(1, 0, 2).reshape(P, BPC * CN * ADIM))
        in_maps.append({
            "WT": np.ascontiguousarray(WT[s]),
            "ETall": et,
            "obsT": np.ascontiguousarray(obsT[:, s]),
            "gbrow": gbrow,
            "cmrow": cmrow,
            "vs0row": vs0row,
            "biascol": bcol,
            "DTall": dtall,
        })
    return in_maps


_NC_CACHE = {}


def _get_nc(cm_const=None):
    key = cm_const
    if key not in _NC_CACHE:
        _NC_CACHE[key] = build_nc(cm_const=cm_const)
    return _NC_CACHE[key]


def _detect_cm_const(tau, mask):
    tau = np.asarray(tau, np.float32)
    mask = np.asarray(mask, np.float32)
    if np.all(mask == 1.0) and np.all(tau == tau.flat[0]):
        return float(1.0 - DT / tau.flat[0])
    return None


def kernel(obs, v0, tau, gain, bias, W, mask, E, D):
    nc = _get_nc(_detect_cm_const(tau, mask))
    in_maps = prep_in_maps(obs, v0, tau, gain, bias, W, mask, E, D)
    res = run_bass_kernel_spmd(nc, in_maps, core_ids=list(range(NCORES)))
    return np.concatenate([res.results[c]["act"] for c in range(NCORES)], axis=0)


# revision 68
# speedup vs baseline: 1.1512x; 1.1512x over previous
"""CTRNN policy kernel for Trainium2 (8 NeuronCores, batch-parallel).

Reference computation (per batch element b, B=64, N=1024, OBS=64, A=16):
    I = E[b] @ obs[b]
    repeat int(1.0//0.1)=9 times:
        y = tanh(gain*(v+bias))*mask
        v = (v + DT/tau * (-v + W[b]@y + I)) * mask
    action[b] = D[b] @ v

Sharding: batch 64 -> 8 cores x 8 individuals, fully data parallel.

Per-core algebra (host-folded, mask/tau folded into the coefficients):
    am = DT/tau*mask, cm = (1-DT/tau)*mask
    Wf = diag(am) @ W @ diag(mask);  Ef = diag(am) @ E;  bc = bias*(1-cm)
    state vs = v + bias:
        y   = tanh(g * vs)
        vs' = cm*vs + Wf@y + (Ef@obs + bc)
    action = D @ (vs - bias)

Device mapping per individual (N=1024 as n = p*8 + c for the matmul
contraction; W^T slabs [128, 8192] bf16 all resident in SBUF):

  - matvec on TensorE with 4-way column-group tiling: stationary = y column
    chunk [128,1] bf16 at array column 32j, moving = W^T n-slab [128,256].
    The 4 groups stream concurrently (separate XBUSes) and land in ONE
    shared PSUM bank at partitions {0,32,64,96} (disjoint per-partition
    accumulators), so a matvec costs ~1.9us of PE instead of ~3.5us.
  - the leak/gate update runs in "row space" [128,256] right out of PSUM
    (rows 32j hold dv n-slab j; other lanes carry zeros): tensor_tensor ops
    are lane-parallel so the garbage lanes are free. The only partition
    redistribution is the y scatter [4x256 rows] -> [128,8] bf16 column
    layout, issued at the END of the chain on the ACT HWDGE ring: its ~1.5us
    DMA completion latency is absorbed by the 6-wide round-robin before the
    same individual's next matvec needs y -- no engine FIFO ever waits on a
    DMA completion (that coupling capped earlier versions at ~2.9us/matvec).
"""

import os
import sys
from contextlib import ExitStack

import numpy as np

for _p in ("/opt/trn_rl_repo", "/root/.axon_site/_ro/trn_rl_repo"):
    if os.path.isdir(_p) and _p not in sys.path:
        sys.path.append(_p)

import ml_dtypes  # noqa: E402

import concourse.bass as bass  # noqa: E402
import concourse.tile as tile  # noqa: E402
from concourse import bacc, mybir  # noqa: E402
from concourse.bass_utils import run_bass_kernel_spmd  # noqa: E402

DT = 0.1
ITERS = int(1.0 // DT)  # == 9: 1.0//0.1 == 9.0 in fp
B_FULL, N, OBS, ADIM = 64, 1024, 64, 16
NCORES = 8
BPC = B_FULL // NCORES
P, CN = 128, 8          # n = p*8 + c
F32 = mybir.dt.float32
BF16 = mybir.dt.bfloat16
BF16_NP = ml_dtypes.bfloat16

CG = 4                  # column groups
NSLAB = N // CG         # 256
GSLABS = [(32 * j, 256 * j) for j in range(CG)]  # (tile_col, w_off)
PSW = 512               # one PSUM bank per matvec
PS_BUFS = 8

# contraction chunk map for the DVE-block-transpose y distribution:
# chunk k's stationary column is yT[:, 32k] where yT = 32x32-block-transpose
# of row-space y4 (rows 32j hold n-slab j). That column holds
# y[m_k(p)] with m_k(p) = 256*(p//32) + 32*k + (p%32); the host permutes the
# W^T slabs to match, so no DMA scatter is needed anywhere in the loop.
_pidx = np.arange(P)
M_INDEX = (256 * (_pidx[:, None] // 32) + 32 * np.arange(CN)[None, :]
           + (_pidx[:, None] % 32))  # [128, 8]


def wave_schedule(iters=ITERS, mv_us=2.0):
    """Greedy longest-queue-first rounds honoring estimated W arrival.

    Returns rounds (lists of (b, t) work items). Width grows as W tiles land
    (~4.8us each after W0+bulk), then stays ~6-wide so the per-individual
    y chain (incl. scatter DMA latency) hides behind other individuals.
    """
    w_avail = [8.0, 16.0, 23.0, 30.0, 37.0, 44.0, 51.0, 58.0]
    t = 9.0
    remaining = [iters] * BPC
    rounds = []
    while any(remaining):
        active = [b for b in range(BPC) if remaining[b] and w_avail[b] <= t]
        if not active:
            t = min(w_avail[b] for b in range(BPC) if remaining[b])
            continue
        active.sort(key=lambda b: (-remaining[b], b))
        active = active[:6]
        rounds.append([(b, iters - remaining[b]) for b in active])
        for b in active:
            remaining[b] -= 1
        t += max(mv_us * len(active), 8.0 if len(active) < 4 else 0.0)
    return rounds


def make_pools(ctx, tc):
    return dict(
        const=ctx.enter_context(tc.tile_pool(name="const", bufs=1)),
        wpool=ctx.enter_context(tc.tile_pool(name="w", bufs=1)),
        et=ctx.enter_context(tc.tile_pool(name="et", bufs=2)),
        row=ctx.enter_context(tc.tile_pool(name="row", bufs=2)),
        tmp=ctx.enter_context(tc.tile_pool(name="tmp", bufs=4)),
        t1p=ctx.enter_context(tc.tile_pool(name="t1p", bufs=8)),
        ps=ctx.enter_context(tc.tile_pool(name="ps", bufs=PS_BUFS, space="PSUM")),
    )


def kernel_body(ctx, tc, ins, out_ap, iters=ITERS, pools=None, cm_const=None):
    nc = tc.nc
    Tanh = mybir.ActivationFunctionType.Tanh
    add = mybir.AluOpType.add
    mult = mybir.AluOpType.mult
    sub = mybir.AluOpType.subtract

    p = pools if pools is not None else make_pools(ctx, tc)
    const, wpool, etp, row, tmp, ps = (
        p["const"], p["wpool"], p["et"], p["row"], p["tmp"], p["ps"])
    t1p = p["t1p"]

    # ---- bulk loads on the SP (sync) HWDGE ring: W first ----
    w_sb = [wpool.tile([P, CN * N], BF16, tag=f"w{b}", name=f"w{b}")
            for b in range(BPC)]
    obs_sb = const.tile([OBS, BPC], BF16, tag="obs", name="obs")
    nc.sync.dma_start(obs_sb[:], ins["obsT"][:])
    bias_c = const.tile([P, BPC * 2 * CN], F32, tag="biasc", name="biasc")
    nc.sync.dma_start(bias_c[:], ins["biascol"][:])
    gbrow = const.tile([P, BPC * 2 * NSLAB], F32, tag="gbrow", name="gbrow")
    nc.sync.dma_start(gbrow[:, :3 * 2 * NSLAB], ins["gbrow"][:, :3 * 2 * NSLAB])
    vs4 = [const.tile([P, NSLAB], BF16, tag=f"vs{b}", name=f"vs{b}")
           for b in range(BPC)]
    for b in range(3):
        nc.sync.dma_start(vs4[b][:], ins["vs0row"][:, b * NSLAB:(b + 1) * NSLAB])
    nc.sync.dma_start(w_sb[0][:], ins["WT"][0])
    nc.sync.dma_start(w_sb[1][:], ins["WT"][1])
    nc.sync.dma_start(gbrow[:, 3 * 2 * NSLAB:], ins["gbrow"][:, 3 * 2 * NSLAB:])
    for b in range(3, BPC):
        nc.sync.dma_start(vs4[b][:], ins["vs0row"][:, b * NSLAB:(b + 1) * NSLAB])
    nc.sync.dma_start(w_sb[2][:], ins["WT"][2])
    cmrow = None
    if cm_const is None:
        cmrow = const.tile([P, BPC * NSLAB], F32, tag="cmrow", name="cmrow")
        nc.sync.dma_start(cmrow[:], ins["cmrow"][:])
    dt_sb = const.tile([P, BPC * CN * ADIM], F32, tag="dt", name="dt")
    nc.sync.dma_start(dt_sb[:], ins["DTall"][:])
    for b in range(3, BPC):
        nc.sync.dma_start(w_sb[b][:], ins["WT"][b])

    i4_sb, y_sb = [], []
    for b in range(BPC):
        i4_sb.append(const.tile([P, NSLAB], F32, tag=f"i4{b}", name=f"i4{b}"))
        y_sb.append(const.tile([P, NSLAB], BF16, tag=f"y{b}", name=f"y{b}"))
    act_sb = const.tile([1, BPC * ADIM], F32, tag="act", name="act")

    def cm4_ap(b):
        return cmrow[:, b * NSLAB:(b + 1) * NSLAB]

    def g4_ap(b):
        return gbrow[:, (b * 2 + 0) * NSLAB:(b * 2 + 1) * NSLAB]

    def bc4_ap(b):
        return gbrow[:, (b * 2 + 1) * NSLAB:(b * 2 + 2) * NSLAB]

    # gate + y distribution: s4(bf16) -> DVE 32x32 block-transpose -> tanh.
    # tanh(transpose(x)) == transpose(tanh(x)), and transposing first keeps
    # the vector->vector hop adjacent with ScalarE writing y_sb[b] directly;
    # yT[:, 32k] is chunk k's stationary column. No DMA anywhere.
    def emit_y(b, s4):
        sT = tmp.tile([P, NSLAB], BF16, tag="y4", name="y4")
        nc.vector.transpose(sT[:], s4[:])
        nc.scalar.activation(y_sb[b][:], sT[:], Tanh)

    # ---- setup: I row = Ef@obs, scatter to row space, add bc; y0 ----
    for b in range(BPC):
        ir = row.tile([1, N], F32, tag="irow", name=f"ir{b}")
        et = etp.tile([OBS, N], BF16, tag="et", name="et")
        # ACT-ring DMA: must not queue behind the W stream on the SP ring
        nc.scalar.dma_start(et[:], ins["ETall"][:, b * N:(b + 1) * N])
        for h in range(2):
            ip = ps.tile([P, PSW], F32, tag="ps", name=f"ip{b}_{h}")
            nc.tensor.matmul(
                ip[0:1, 0:512],
                obs_sb[:, b:b + 1],
                et[:, h * 512:(h + 1) * 512],
                start=True, stop=True,
            )
            nc.scalar.copy(ir[0:1, h * 512:(h + 1) * 512], ip[0:1, 0:512])
        nc.scalar.dma_start(i4_sb[b][0:P:32, :], ir[:])  # [1,1024]->[4,256] rows
        nc.vector.tensor_tensor(i4_sb[b][:], i4_sb[b][:], bc4_ap(b), op=add)
        emit_y(b, vs4[b])

    # ---- recurrent loop ----
    t1_sb, wy_sb = {}, {}

    def matvec_mm(b, t):
        # t1 = cm*vs + I only needs last iteration's vs -- runs during the
        # matmuls, off the post-matmul chain. With constant tau/mask the
        # leak multiply is a ScalarE const-mul, keeping VectorE under PE.
        tg = tmp.tile([P, NSLAB], F32, tag="tg", name="tg")
        if cm_const is not None:
            nc.scalar.mul(tg[:], vs4[b][:], cm_const)
        else:
            nc.vector.tensor_tensor(tg[:], cm4_ap(b), vs4[b][:], op=mult)
        t1 = t1p.tile([P, NSLAB], F32, tag="t1", name="t1")
        nc.vector.tensor_tensor(t1[:], tg[:], i4_sb[b][:], op=add)
        t1_sb[b] = t1
        wy = ps.tile([P, PSW], F32, tag="ps", name="wy")
        wy_sb[b] = wy
        for c in range(CN):
            yc = y_sb[b][:, 32 * c:32 * c + 1]
            for (tcol, woff) in GSLABS:
                nc.tensor.matmul(
                    wy[tcol:tcol + 1, 0:NSLAB],
                    yc,
                    w_sb[b][:, c * N + woff: c * N + woff + NSLAB],
                    start=(c == 0), stop=(c == CN - 1),
                    tile_position=(0, tcol),
                )

    def matvec_upd(b, t):
        # row-space leak/gate update straight out of PSUM
        nc.vector.tensor_tensor(vs4[b][:], wy_sb[b][:, 0:NSLAB], t1_sb[b][:],
                                op=add)
        if t < iters - 1:
            emit_y(b, vs4[b])

    # ---- decode: action = D @ (vs - bias) ----
    vcol_sb = {}

    def decode_pre(b):
        # issued right after b's last update; completes long before the
        # end-of-kernel decode matmuls so they never stall PE
        vcol = const.tile([P, CN], BF16, tag=f"vc{b}", name=f"vc{b}")
        nc.scalar.dma_start(vcol[:], vs4[b][0:P:32, :])
        vcol_sb[b] = vcol

    def decode(b):
        vg = tmp.tile([P, CN], F32, tag="vg", name="vg")
        nc.vector.tensor_tensor(
            vg[:], vcol_sb[b][:], bias_c[:, 2 * b * CN:(2 * b + 1) * CN],
            op=mult)
        vf = tmp.tile([P, CN], F32, tag="vf", name="vf")
        nc.vector.tensor_tensor(
            vf[:], vg[:], bias_c[:, (2 * b + 1) * CN:(2 * b + 2) * CN], op=sub)
        ap = ps.tile([P, PSW], F32, tag="ps", name="dec")
        for c in range(CN):
            nc.tensor.matmul(
                ap[0:1, 0:ADIM],
                vf[:, c:c + 1],
                dt_sb[:, b * CN * ADIM + c * ADIM: b * CN * ADIM + (c + 1) * ADIM],
                start=(c == 0), stop=(c == CN - 1),
            )
        nc.vector.tensor_copy(act_sb[0:1, b * ADIM:(b + 1) * ADIM], ap[0:1, 0:ADIM])

    for round_items in wave_schedule(iters):
        for b, t in round_items:
            matvec_mm(b, t)
        for b, t in round_items:
            matvec_upd(b, t)
            if t == iters - 1:
                decode_pre(b)
    for b in range(BPC):
        decode(b)
    nc.sync.dma_start(out_ap[:], act_sb[0:1, :])


def build_nc(iters=ITERS, cm_const=None):
    nc = bacc.Bacc(
        "TRN2", target_bir_lowering=False, debug=False, enable_asserts=False,
    )
    ins = {}
    ins["WT"] = nc.dram_tensor("WT", [BPC, P, CN * N], BF16, kind="ExternalInput").ap()
    ins["ETall"] = nc.dram_tensor("ETall", [OBS, BPC * N], BF16, kind="ExternalInput").ap()
    ins["obsT"] = nc.dram_tensor("obsT", [OBS, BPC], BF16, kind="ExternalInput").ap()
    ins["gbrow"] = nc.dram_tensor(
        "gbrow", [P, BPC * 2 * NSLAB], F32, kind="ExternalInput").ap()
    ins["cmrow"] = nc.dram_tensor(
        "cmrow", [P, BPC * NSLAB], F32, kind="ExternalInput").ap()
    ins["vs0row"] = nc.dram_tensor(
        "vs0row", [P, BPC * NSLAB], BF16, kind="ExternalInput").ap()
    ins["biascol"] = nc.dram_tensor(
        "biascol", [P, BPC * 2 * CN], F32, kind="ExternalInput").ap()
    ins["DTall"] = nc.dram_tensor(
        "DTall", [P, BPC * CN * ADIM], F32, kind="ExternalInput").ap()
    out_ap = nc.dram_tensor("act", [BPC, ADIM], F32, kind="ExternalOutput").ap()

    with tile.TileContext(nc) as tc:
        with ExitStack() as ctx:
            pools = make_pools(ctx, tc)
            kernel_body(ctx, tc, ins, out_ap, iters, pools, cm_const)
    nc.compile()
    return nc


def _to_rowspace(arr):
    """[B, N] -> [B, 128, NSLAB] row-space: row 32j holds n-slab j, rest 0."""
    B = arr.shape[0]
    out = np.zeros((B, P, NSLAB), np.float32)
    for j in range(CG):
        out[:, 32 * j, :] = arr[:, NSLAB * j:NSLAB * (j + 1)]
    return out


def prep_in_maps(obs, v0, tau, gain, bias, W, mask, E, D):
    f = np.float32
    obs, v0, tau, gain, bias, W, mask, E, D = [
        np.asarray(x, dtype=f) for x in (obs, v0, tau, gain, bias, W, mask, E, D)
    ]
    am = (DT / tau) * mask                    # [64, N]
    cm = (1.0 - DT / tau) * mask
    Wf = W * (am * gain)[:, :, None] * mask[:, None, :]
    # permute the contraction dim to the block-transpose chunk map:
    # WT[b, p, k*N + n] = Wf[b, n, M_INDEX[p, k]]   (done per-core for memory)
    WT = np.empty((B_FULL, P, CN * N), BF16_NP)
    for b in range(B_FULL):
        WT[b] = Wf[b][:, M_INDEX].transpose(1, 2, 0).reshape(
            P, CN * N).astype(BF16_NP)
    ETp = np.ascontiguousarray(
        (E * (am * gain)[:, :, None]).transpose(0, 2, 1)).astype(BF16_NP)
    DTp = np.ascontiguousarray(D.transpose(0, 2, 1)).reshape(B_FULL, P, CN * ADIM)
    obsT = np.ascontiguousarray(obs.T).astype(BF16_NP)  # [OBS, 64]
    cm4 = _to_rowspace(cm)
    g4 = _to_rowspace(gain)
    bc4 = _to_rowspace(gain * bias * (1.0 - cm))
    vs04 = _to_rowspace(gain * (v0 + bias))
    invg = np.where(gain != 0.0, 1.0 / np.maximum(gain, 1e-30), 0.0)
    biascol = np.concatenate(
        [invg.reshape(B_FULL, P, 1, CN), bias.reshape(B_FULL, P, 1, CN)],
        axis=2).reshape(B_FULL, P, 2 * CN)

    in_maps = []
    for core in range(NCORES):
        s = slice(core * BPC, (core + 1) * BPC)
        gbrow = np.empty((P, BPC * 2 * NSLAB), f)
        cmrow = np.empty((P, BPC * NSLAB), f)
        for i, b in enumerate(range(core * BPC, (core + 1) * BPC)):
            for k, arr in enumerate((g4, bc4)):
                gbrow[:, (i * 2 + k) * NSLAB:(i * 2 + k + 1) * NSLAB] = arr[b]
            cmrow[:, i * NSLAB:(i + 1) * NSLAB] = cm4[b]
        vs0row = np.ascontiguousarray(
            vs04[s].transpose(1, 0, 2).reshape(P, BPC * NSLAB)).astype(BF16_NP)
        bcol = np.ascontiguousarray(
            biascol[s].transpose(1, 0, 2).reshape(P, BPC * 2 * CN))
        et = np.ascontiguousarray(
            ETp[s].transpose(1, 0, 2).reshape(OBS, BPC * N))
        dtall = np.ascontiguousarray(
            DTp[s].transpose(1, 0, 2).reshape(P, BPC * CN * ADIM))
        in_maps.append({
            "WT": np.ascontiguousarray(WT[s]),
            "ETall": et,
            "obsT": np.ascontiguousarray(obsT[:, s]),
            "gbrow": gbrow,
            "cmrow": cmrow,
            "vs0row": vs0row,
            "biascol": bcol,
            "DTall": dtall,
        })
    return in_maps


_NC_CACHE = {}


def _get_nc(cm_const=None):
    key = cm_const
    if key not in _NC_CACHE:
        _NC_CACHE[key] = build_nc(cm_const=cm_const)
    return _NC_CACHE[key]


def _detect_cm_const(tau, mask):
    tau = np.asarray(tau, np.float32)
    mask = np.asarray(mask, np.float32)
    if np.all(mask == 1.0) and np.all(tau == tau.flat[0]):
        return float(1.0 - DT / tau.flat[0])
    return None


def kernel(obs, v0, tau, gain, bias, W, mask, E, D):
    nc = _get_nc(_detect_cm_const(tau, mask))
    in_maps = prep_in_maps(obs, v0, tau, gain, bias, W, mask, E, D)
    res = run_bass_kernel_spmd(nc, in_maps, core_ids=list(range(NCORES)))
    return np.concatenate([res.results[c]["act"] for c in range(NCORES)], axis=0)
